# revision 1
# baseline (speedup 1.0000x reference)
"""Trainium2 Bass kernel: CenterHeadIoU 1x1-conv stack.

Computes, for x = ct_feat [B=32, C=128, N=8192]:
  y = relu(bn(sh_w @ x))                       [B, 64, N]
  z_h = relu(bn_h(head_w1[h] @ y)), h=0..5     [B, 64, N] each
  out = concat_h(head_final_w[h] @ z_h + b_h)  [B, 12, N]

Sharding: data-parallel over batch, 4 batches per core on 8 cores;
weights are tiny and replicated. BN is folded into conv weights/biases
on the host. On device, per 512-column tile:
  mm1: lhsT [128,64] -> psum y [64, F]
  act1: relu(y + b1) PSUM->SBUF into y65[0:64] (row 64 holds constant 1.0)
  mm2 (x3): K=65 full-density pair weights with a bias row -> one
       [128,1536] psum tensor (chunk p = heads 2p/2p+1 stacked)
  z relu: one ACT op on [128,1024] + one DVE op on [128,512] (bias-free)
  mm3 (x3): accumulating matmuls (M=12, zero-padded pair blocks) into a
       dense [12, F] psum bank
  epi: DVE bias-add into a dense [12, N] per-batch accumulator; one DMA
       out per batch.
All matmuls run as float32r (full-rate fp32 mode, free dim 512).
A post-pass moves multi-wait sync conditions onto single-wait NoOp
carriers (this walrus build caps sync waits per instruction).
"""

import os
import sys
import numpy as np

B, C_IN, N, HC = 32, 128, 8192, 64
NCORES = 8
BC = B // NCORES            # batches per core
F = 512                     # free-dim tile = one fp32 PSUM bank
NT = N // F                 # tiles per batch
EPS = 1e-5
HEAD_OUT = [3, 2, 1, 3, 2, 1]        # hm, reg, height, dim, rot, iou
PAIR_OFF = [0, 5, 9]                 # channel offset of pair p in the 12-ch output

_CACHE = {}
LAST_RESULTS = None
LAST_EXEC_NS = None


def _build_program():
    import concourse.bass as bass
    import concourse.mybir as mybir
    import concourse.tile as tile

    f32 = mybir.dt.float32
    f32r = mybir.dt.float32r
    AF = mybir.ActivationFunctionType

    nc = bass.Bass("TRN2", target_bir_lowering=False, debug=False,
                   num_devices=NCORES)

    x = nc.dram_tensor("x", [BC, C_IN, N], f32r, kind="ExternalInput").ap()
    w1 = nc.dram_tensor("w1", [C_IN, HC], f32r, kind="ExternalInput").ap()
    b1 = nc.dram_tensor("b1", [HC, 1], f32, kind="ExternalInput").ap()
    w2 = nc.dram_tensor("w2", [HC + 1, 384], f32r, kind="ExternalInput").ap()
    w3 = nc.dram_tensor("w3", [128, 48], f32r, kind="ExternalInput").ap()
    b3 = nc.dram_tensor("b3", [12, 1], f32, kind="ExternalInput").ap()
    ones = nc.dram_tensor("ones", [1, F], f32r, kind="ExternalInput").ap()
    out = nc.dram_tensor("out", [BC, 12, N], f32, kind="ExternalOutput").ap()

    with tile.TileContext(nc) as tc:
        with (
            tc.tile_pool(name="consts", bufs=1) as cpool,
            tc.tile_pool(name="xin", bufs=4) as xpool,
            tc.tile_pool(name="ysb", bufs=1) as ypool,
            tc.tile_pool(name="zsb", bufs=2) as zpool,
            tc.tile_pool(name="osb", bufs=2) as opool,
            tc.tile_pool(name="py", bufs=1, space="PSUM") as pypool,
            tc.tile_pool(name="pz", bufs=2, space="PSUM") as pzpool,
            tc.tile_pool(name="po", bufs=1, space="PSUM") as popool,
        ):
            w1_t = cpool.tile([C_IN, HC], f32r, name="w1_t")
            nc.sync.dma_start(out=w1_t[:], in_=w1[:])
            b1_t = cpool.tile([HC, 1], f32, name="b1_t")
            nc.sync.dma_start(out=b1_t[:], in_=b1[:])
            w2_t = cpool.tile([HC + 1, 384], f32r, name="w2_t")
            nc.sync.dma_start(out=w2_t[:], in_=w2[:])
            w3_t = cpool.tile([128, 48], f32r, name="w3_t")
            nc.sync.dma_start(out=w3_t[:], in_=w3[:])
            b3_t = cpool.tile([12, 1], f32, name="b3_t")
            nc.sync.dma_start(out=b3_t[:], in_=b3[:])

            # y tiles hoisted: row 64 stays constant 1.0 (feeds the bias row
            # of the K=65 mm2), rows 0..63 rewritten by act1 each iteration.
            y_tiles = []
            for i in range(3):
                y65 = ypool.tile([HC + 1, F], f32r, name=f"y65_{i}",
                                 tag=f"y65_{i}")
                nc.sync.dma_start(out=y65[HC:HC + 1, :], in_=ones[:])
                y_tiles.append(y65)

            T = BC * NT

            def load_x(t):
                b, j = divmod(t, NT)
                xt = xpool.tile([C_IN, F], f32r, name="xt", tag="xt")
                nc.sync.dma_start(out=xt[:], in_=x[b, :, j * F:(j + 1) * F])
                return xt

            def mm1_act1(t, xt):
                py = pypool.tile([HC, F], f32, name="py", tag="py")
                nc.tensor.matmul(py[:], w1_t[:], xt[:], start=True, stop=True)
                y65 = y_tiles[t % 3]
                nc.scalar.activation(y65[0:HC, :], py[:], AF.Relu,
                                     bias=b1_t[:, 0:1], scale=1.0)
                return y65

            def mm3_epi(t, zt, ots):
                b, j = divmod(t, NT)
                po = popool.tile([12, F], f32, name="po", tag="po")
                for p in range(3):
                    nc.tensor.matmul(po[:, :],
                                     w3_t[:, 12 * p:12 * (p + 1)],
                                     zt[:, F * p:F * (p + 1)],
                                     start=(p == 0), stop=(p == 2))
                ot = ots[t]
                nc.vector.tensor_scalar_add(ot[:, j * F:(j + 1) * F],
                                            po[:, :], b3_t[:, 0:1])
                if j == NT - 1:
                    nc.sync.dma_start(out=out[b, :, :], in_=ot[:, :])

            # two-deep software pipeline: cycle t runs mm3/epi of tile t-1,
            # mm2/relu of tile t, and mm1/act1 of tile t+1, so the PE never
            # waits on the current tile's relu results.
            ots = {}
            xt = load_x(0)
            y65 = mm1_act1(0, xt)
            z_prev = None
            for t in range(T):
                b, j = divmod(t, NT)
                if j == 0:
                    ot = opool.tile([12, N], f32, name="ot", tag="ot")
                for jj in range(j, NT):
                    ots[b * NT + jj] = ot
                if t + 1 < T:
                    xt_next = load_x(t + 1)

                if z_prev is not None:
                    mm3_epi(t - 1, z_prev, ots)

                pz = pzpool.tile([128, 3 * F], f32, name="pz", tag="pz")
                for p in range(3):
                    nc.tensor.matmul(pz[:, F * p:F * (p + 1)],
                                     w2_t[:, 128 * p:128 * (p + 1)],
                                     y65[:, :],
                                     start=True, stop=True)
                zt = zpool.tile([128, 3 * F], f32r, name="zt", tag="zt")
                # bias already folded into mm2; pure relu
                nc.scalar.activation(zt[:, 0:2 * F], pz[:, 0:2 * F], AF.Relu)
                nc.vector.tensor_scalar_max(zt[:, 2 * F:3 * F],
                                            pz[:, 2 * F:3 * F], 0.0)

                if t + 1 < T:
                    y65 = mm1_act1(t + 1, xt_next)
                z_prev = zt
            mm3_epi(T - 1, z_prev, ots)
    _split_waits(nc)
    return nc


def _split_waits(nc, cap=1):
    """This container's walrus build rejects instructions carrying more than
    a small number of sync waits (fp32/f32r matmuls: just one). Move excess
    waits onto single-wait NoOp carriers inserted before the instruction on
    the same engine — semantically identical (conjunction of waits, in-order
    sequencers)."""
    import concourse.mybir as mybir

    k = 0
    for func in nc.m.functions:
        for bb in func.blocks:
            insts = bb.instructions
            out_insts = []
            changed = False
            for inst in insts:
                si = inst.sync_info
                waits = list(si.on_wait) if si and si.on_wait else []
                if len(waits) > cap:
                    for w in waits[:-cap]:
                        d = mybir.InstNoOp(name=f"I-sw{k}", ins=[], outs=[])
                        k += 1
                        d.engine = inst.engine
                        d.sync_info = mybir.SyncInfo(on_wait=[w], on_update=[])
                        nc.register_instruction(d)
                        out_insts.append(d)
                    inst.sync_info = mybir.SyncInfo(
                        on_wait=waits[-cap:],
                        on_update=list(si.on_update) if si.on_update else [])
                    changed = True
                out_insts.append(inst)
            if changed:
                bb.instructions = out_insts


def _get_program():
    if "nc" not in _CACHE:
        _CACHE["nc"] = _build_program()
    return _CACHE["nc"]


def _prep_weights(d):
    """Fold BN into conv weights/biases; pack stationary matrices."""
    f8 = np.float64

    def g(name):
        return np.asarray(d[name], dtype=f8)

    # shared conv + BN
    s1 = g("sh_g") / np.sqrt(g("sh_var") + EPS)                     # [64]
    W1e = g("sh_w") * s1[:, None]                                   # [64,128]
    b1e = g("sh_b") * s1 + g("sh_beta") - g("sh_mean") * s1         # [64]
    w1 = W1e.T.copy()                                               # [128,64]
    b1 = b1e[:, None]                                               # [64,1]

    # head first layers + BN: K=65 stationaries with a bias row; pair p's
    # block has head 2p in output cols 0..63 and head 2p+1 in cols 64..127.
    s2 = g("head_g1") / np.sqrt(g("head_var1") + EPS)               # [6,64]
    W2e = g("head_w1") * s2[:, :, None]                             # [6,64,64]
    b2e = g("head_b1") * s2 + g("head_beta1") - g("head_mean1") * s2  # [6,64]
    w2 = np.zeros((HC + 1, 384), f8)
    for p in range(3):
        w2[0:HC, 128 * p:128 * p + 64] = W2e[2 * p].T
        w2[0:HC, 128 * p + 64:128 * p + 128] = W2e[2 * p + 1].T
        w2[HC, 128 * p:128 * p + 64] = b2e[2 * p]
        w2[HC, 128 * p + 64:128 * p + 128] = b2e[2 * p + 1]

    # final convs: three accumulating M=12 blocks
    names = ["hm", "reg", "height", "dim", "rot", "iou"]
    Wf = [g(n + "_w") for n in names]
    bf = [g(n + "_b") for n in names]
    w3 = np.zeros((128, 48), f8)
    b3 = np.zeros((12, 1), f8)
    for p in range(3):
        ha, hb = 2 * p, 2 * p + 1
        ca, cb = HEAD_OUT[ha], HEAD_OUT[hb]
        off = PAIR_OFF[p]
        w3[0:64, 12 * p + off:12 * p + off + ca] = Wf[ha].T
        w3[64:128, 12 * p + off + ca:12 * p + off + ca + cb] = Wf[hb].T
        b3[off:off + ca, 0] = bf[ha]
        b3[off + ca:off + ca + cb, 0] = bf[hb]

    c = np.float32
    return {"w1": w1.astype(c), "b1": b1.astype(c), "w2": w2.astype(c),
            "w3": w3.astype(c), "b3": b3.astype(c),
            "ones": np.ones((1, F), np.float32)}


def _ensure_ntff_hook():
    """Install the antenv.axon_hooks NTFF-profile shim if the container's
    antenv package lacks it (profiling only; never used in grading runs)."""
    try:
        from antenv.axon_hooks import get_axon_ntff_profile_hook  # noqa: F401
        return True
    except ImportError:
        pass
    import contextlib
    import ctypes
    import sys as _sys
    import types

    so_path = "/opt/axon/libaxon_pjrt.so"
    if not os.path.exists(so_path):
        return False
    lib = ctypes.CDLL(so_path)
    if not hasattr(lib, "axon_start_nrt_profile"):
        return False
    lib.axon_start_nrt_profile.argtypes = [ctypes.POINTER(ctypes.c_int64),
                                           ctypes.c_size_t]
    lib.axon_start_nrt_profile.restype = ctypes.c_int64
    lib.axon_stop_nrt_profile.argtypes = [ctypes.c_char_p]
    lib.axon_stop_nrt_profile.restype = ctypes.c_int64

    @contextlib.contextmanager
    def _hook(output_dir, device_ids):
        import jax
        jax.devices()
        if device_ids:
            ids = (ctypes.c_int64 * len(device_ids))(*device_ids)
            rc = lib.axon_start_nrt_profile(ids, len(device_ids))
        else:
            rc = lib.axon_start_nrt_profile(None, 0)
        if rc != 0:
            raise RuntimeError(f"axon_start_nrt_profile rc={rc}")
        try:
            yield
        finally:
            n = lib.axon_stop_nrt_profile(str(output_dir).encode())
            print(f"profile: {n} file(s) written to {output_dir}",
                  file=sys.stderr)

    import antenv
    mod = types.ModuleType("antenv.axon_hooks")
    mod.get_axon_ntff_profile_hook = lambda: _hook
    mod.set_axon_ntff_profile_hook = lambda h: None
    _sys.modules["antenv.axon_hooks"] = mod
    antenv.axon_hooks = mod
    return True


def kernel(**inputs):
    global LAST_RESULTS, LAST_EXEC_NS
    from concourse.bass_utils import run_bass_kernel_spmd

    inputs = {k: np.asarray(v) for k, v in inputs.items()}
    weights = _prep_weights(inputs)

    ct = np.asarray(inputs["ct_feat"], dtype=np.float32)
    xs = ct.reshape(NCORES, BC, C_IN, N)

    in_maps = [dict(weights, x=np.ascontiguousarray(xs[i]))
               for i in range(NCORES)]

    nc = _get_program()
    trace = bool(int(os.environ.get("CK_PROFILE", "0")))
    if trace:
        trace = _ensure_ntff_hook()
    res = run_bass_kernel_spmd(nc, in_maps, list(range(NCORES)), trace=trace)
    LAST_RESULTS = res
    LAST_EXEC_NS = res.exec_time_ns

    out = np.concatenate([np.asarray(res.results[i]["out"])
                          for i in range(NCORES)], axis=0)
    return out.astype(np.float32)



# revision 20
# speedup vs baseline: 2.0740x; 2.0740x over previous
"""Trainium2 Bass kernel: CenterHeadIoU 1x1-conv stack.

Computes, for x = ct_feat [B=32, C=128, N=8192]:
  y = relu(bn(sh_w @ x))                       [B, 64, N]
  z_h = relu(bn_h(head_w1[h] @ y)), h=0..5     [B, 64, N] each
  out = concat_h(head_final_w[h] @ z_h + b_h)  [B, 12, N]

Sharding: data-parallel over batch, 4 batches per core on 8 cores;
weights are tiny and replicated. BN is folded into conv weights/biases
on the host; all matmul operands are bf16 (PSUM accumulation stays
fp32, and the rel-err budget of 2e-2 leaves bf16 plenty of margin).

Per 512-column tile the PE runs exactly 7 x 512-row matmuls:
  mm1: w1 [128,64] @ x tile            -> py   [64,512]
  mm2: 3x w2 pair blocks (K=65, the 65th row carries the folded bias
       against a constant-1.0 row of y)  -> pza [128,1024] + pzb [128,512]
  mm3: 3x accumulating pair blocks with slot-expanded lhsT [128,48]
       (the live 12 rows sit at partition offset 12*(t%4), rest zeros)
       so FOUR tiles' outputs land stacked in ONE psum bank [48,512].
Epilogue is then one ACT Identity(+bias) op per 4 tiles and one striped
DMA straight to DRAM (partition 12s+c -> out[b, c, (j0+s)*512 + f]).

The loop is software-pipelined two tiles deep (mm3 consumes z from two
tiles back) so every PE matmul's dependencies resolve at least a full
tile early: the PE never waits on ACT/DVE, runs back-to-back, and the
HAM clock gate holds it at 2.4 GHz.

Elementwise work is split so both PSUM-capable engines stay under the
PE's ~1.49us/tile: ACT does act1 + the [128,512] z chunk + the per-4-
tile epi (~1.31us/tile); DVE does the [128,1024] z chunk (~1.19us).

A post-pass moves multi-wait sync conditions onto single-wait NoOp
carriers (this walrus build caps sync waits per instruction).
"""

import os
import sys
import numpy as np

B, C_IN, N, HC = 32, 128, 8192, 64
NCORES = 8
BC = B // NCORES            # batches per core
F = 512                     # free-dim tile = one fp32 PSUM bank
NT = N // F                 # tiles per batch
T = BC * NT                 # tiles per core
GS = 8                      # tiles per mm3 psum group (slots)
EPS = 1e-5
HEAD_OUT = [3, 2, 1, 3, 2, 1]        # hm, reg, height, dim, rot, iou
PAIR_OFF = [0, 5, 9]                 # channel offset of pair p in the 12-ch output

_CACHE = {}
LAST_RESULTS = None
LAST_EXEC_NS = None


def _build_program():
    import concourse.bass as bass
    import concourse.mybir as mybir
    import concourse.tile as tile

    f32 = mybir.dt.float32
    bf16 = mybir.dt.bfloat16
    AF = mybir.ActivationFunctionType

    nc = bass.Bass("TRN2", target_bir_lowering=False, debug=False,
                   num_devices=NCORES)

    x = nc.dram_tensor("x", [BC, C_IN, N], bf16, kind="ExternalInput").ap()
    w1 = nc.dram_tensor("w1", [C_IN, HC], bf16, kind="ExternalInput").ap()
    b1 = nc.dram_tensor("b1", [HC, 1], f32, kind="ExternalInput").ap()
    w2 = nc.dram_tensor("w2", [HC + 1, 384], bf16, kind="ExternalInput").ap()
    w3s = nc.dram_tensor("w3s", [128, GS * 3 * (12 * GS)], bf16,
                         kind="ExternalInput").ap()
    b3x4 = nc.dram_tensor("b3x4", [GS * 12, 1], f32, kind="ExternalInput").ap()
    ones = nc.dram_tensor("ones", [1, F], bf16, kind="ExternalInput").ap()
    out = nc.dram_tensor("out", [BC, 12, N], f32, kind="ExternalOutput").ap()

    with tile.TileContext(nc) as tc:
        with (
            tc.tile_pool(name="consts", bufs=1) as cpool,
            tc.tile_pool(name="xin", bufs=4) as xpool,
            tc.tile_pool(name="ysb", bufs=1) as ypool,
            tc.tile_pool(name="zsb", bufs=4) as zpool,
            tc.tile_pool(name="esb", bufs=2) as epool,
            tc.tile_pool(name="ppy", bufs=2, space="PSUM") as pypool,
            tc.tile_pool(name="pza", bufs=2, space="PSUM") as pzapool,
            tc.tile_pool(name="pzb", bufs=1, space="PSUM") as pzbpool,
            tc.tile_pool(name="ppo", bufs=1, space="PSUM") as popool,
        ):
            NP = T // 2                 # x is DMA'd in 2-tile pairs

            x_tiles = {}

            def load_x(k, split=False):
                b, j2 = divmod(k, NT // 2)
                xt = xpool.tile([C_IN, 2 * F], bf16, name="xt", tag="xt")
                if split:
                    # two DMAs so mm1 of the first tile only waits on half
                    nc.sync.dma_start(out=xt[:, 0:F],
                                      in_=x[b, :, j2 * 2 * F:j2 * 2 * F + F])
                    nc.sync.dma_start(out=xt[:, F:2 * F],
                                      in_=x[b, :, j2 * 2 * F + F:(j2 + 1) * 2 * F])
                else:
                    nc.sync.dma_start(out=xt[:],
                                      in_=x[b, :, j2 * 2 * F:(j2 + 1) * 2 * F])
                x_tiles[k] = xt

            # DMA order is latency-ordered: the operands of the first few
            # tiles (x pair 0, w1, b1) go first so mm1(0) can start early;
            # mm3's tables are only needed ~8 tiles in.
            load_x(0, split=True)
            w1_t = cpool.tile([C_IN, HC], bf16, name="w1_t")
            nc.sync.dma_start(out=w1_t[:], in_=w1[:])
            b1_t = cpool.tile([HC, 1], f32, name="b1_t")
            nc.sync.dma_start(out=b1_t[:], in_=b1[:])
            w2_t = cpool.tile([HC + 1, 384], bf16, name="w2_t")
            nc.sync.dma_start(out=w2_t[:], in_=w2[:])

            # y tiles hoisted: row 64 stays constant 1.0 (feeds the bias row
            # of the K=65 mm2), rows 0..63 rewritten by act1 each iteration.
            y_tiles = []
            for i in range(3):
                y65 = ypool.tile([HC + 1, F], bf16, name=f"y65_{i}",
                                 tag=f"y65_{i}")
                nc.sync.dma_start(out=y65[HC:HC + 1, :], in_=ones[:])
                y_tiles.append(y65)

            for k in range(1, 4):
                load_x(k)
            w3s_t = cpool.tile([128, GS * 3 * (12 * GS)], bf16, name="w3s_t")
            nc.sync.dma_start(out=w3s_t[:], in_=w3s[:])
            b3x4_t = cpool.tile([GS * 12, 1], f32, name="b3x4_t")
            nc.sync.dma_start(out=b3x4_t[:], in_=b3x4[:])

            def mm1_act1(t):
                xt = x_tiles[t // 2]
                xs = xt[:, (t % 2) * F:(t % 2 + 1) * F]
                py = pypool.tile([HC, F], f32, name="py", tag="py")
                nc.tensor.matmul(py[:], w1_t[:], xs, start=True, stop=True)
                y65 = y_tiles[t % 3]
                nc.scalar.activation(y65[0:HC, :], py[:], AF.Relu,
                                     bias=b1_t[:, 0:1], scale=1.0)

            z_tiles = {}

            def mm2_relu(t):
                y65 = y_tiles[t % 3]
                pza = pzapool.tile([128, 2 * F], f32, name="pza", tag="pza")
                pzb = pzbpool.tile([128, F], f32, name="pzb", tag="pzb")
                for p in range(2):
                    nc.tensor.matmul(pza[:, F * p:F * (p + 1)],
                                     w2_t[:, 128 * p:128 * (p + 1)],
                                     y65[:, :], start=True, stop=True)
                nc.tensor.matmul(pzb[:, :], w2_t[:, 256:384], y65[:, :],
                                 start=True, stop=True)
                za = zpool.tile([128, 2 * F], bf16, name="za", tag="za")
                zb = zpool.tile([128, F], bf16, name="zb", tag="zb")
                nc.vector.tensor_scalar_max(za[:, :], pza[:, :], 0.0)
                nc.scalar.activation(zb[:, :], pzb[:, :], AF.Relu)
                z_tiles[t] = (za, zb)

            po_tiles = {}

            def mm3(u):
                s = u % GS
                g = u // GS
                if s == 0:
                    po_tiles[g] = popool.tile([GS * 12, F], f32, name="po",
                                              tag="po")
                po = po_tiles[g]
                za, zb = z_tiles[u]
                rhs = [za[:, 0:F], za[:, F:2 * F], zb[:, :]]
                W = 12 * GS
                for p in range(3):
                    c0 = W * (3 * s + p)
                    nc.tensor.matmul(po[0:GS * 12, :],
                                     w3s_t[:, c0:c0 + W], rhs[p],
                                     start=(s == 0 and p == 0),
                                     stop=(s == GS - 1 and p == 2),
                                     skip_group_check=True)
                del z_tiles[u]

            def epi_out(g):
                po = po_tiles.pop(g)
                e = epool.tile([GS * 12, F], f32, name="e", tag="e")
                nc.scalar.activation(e[0:GS * 12, :], po[0:GS * 12, :],
                                     AF.Identity, bias=b3x4_t[:, 0:1],
                                     scale=1.0)
                b, g4 = divmod(g, NT // GS)
                j0 = g4 * GS
                dview = out[b, :, j0 * F:(j0 + GS) * F]
                dview = dview.rearrange("c (s f) -> s c f", s=GS)
                nc.sync.dma_start(out=dview, in_=e[0:GS * 12, :])

            # Pre-warm: a few dummy matmuls on a memset scratch tile keep
            # the PE busy while the first x tiles are still in flight — the
            # HAM clock gate releases 2.4 GHz only after sustained activity,
            # so starting that clock early moves the fast-clock flip earlier.
            scr = cpool.tile([C_IN, F], bf16, name="scr")
            nc.vector.memset(scr[:], 0.0)
            for i in range(14):
                pd = pypool.tile([HC, F], f32, name="pd", tag="py")
                nc.tensor.matmul(pd[:], scr[:, 0:HC], scr[:],
                                 start=True, stop=True)

            # Prologue: prime y for tiles 0-1 (x pairs 0-3 already in
            # flight); the loop then runs mm1 two tiles ahead so all its
            # semaphores fire well before the PE sequencer decodes it.
            mm1_act1(0)
            mm1_act1(1)

            # Two-tile-deep software pipeline: iteration t runs mm1/act1 of
            # t+1, mm2/relu of t, and mm3 of t-2, so every PE matmul's
            # inputs were produced at least one full tile earlier.
            def post_mm3(u):
                mm3(u)
                if u % GS == GS - 1:
                    epi_out(u // GS)

            for t in range(T):
                if t % 2 == 0 and t // 2 + 4 < NP:
                    load_x(t // 2 + 4)
                if t + 2 < T:
                    mm1_act1(t + 2)
                mm2_relu(t)
                if t >= 2:
                    post_mm3(t - 2)
            for u in (T - 2, T - 1):
                post_mm3(u)
    _split_waits(nc)
    return nc


def _split_waits(nc, cap=1):
    """This container's walrus build rejects instructions carrying more than
    a small number of sync waits (matmuls: just one). Move excess waits onto
    single-wait NoOp carriers inserted before the instruction on the same
    engine — semantically identical (conjunction of waits, in-order
    sequencers)."""
    import concourse.mybir as mybir

    k = 0
    for func in nc.m.functions:
        for bb in func.blocks:
            insts = bb.instructions
            out_insts = []
            changed = False
            for inst in insts:
                si = inst.sync_info
                waits = list(si.on_wait) if si and si.on_wait else []
                if len(waits) > cap:
                    for w in waits[:-cap]:
                        d = mybir.InstNoOp(name=f"I-sw{k}", ins=[], outs=[])
                        k += 1
                        d.engine = inst.engine
                        d.sync_info = mybir.SyncInfo(on_wait=[w], on_update=[])
                        nc.register_instruction(d)
                        out_insts.append(d)
                    inst.sync_info = mybir.SyncInfo(
                        on_wait=waits[-cap:],
                        on_update=list(si.on_update) if si.on_update else [])
                    changed = True
                out_insts.append(inst)
            if changed:
                bb.instructions = out_insts
